# revision 1
# baseline (speedup 1.0000x reference)
"""DeepseekV2 MoE layer (M=1024, H=1024, N=1024, E=16, top-6 of 8 groups x2)
on 8 Trainium2 NeuronCores.

Sharding: expert parallelism with group-aligned placement. E=16 experts in 8
groups of 2; grouped_topk keeps the top-3 groups and top_k=6 = 3*2 takes ALL
experts of those groups. Core c owns group c (experts 2c, 2c+1), so both its
experts process exactly the same token subset: one gather, shared PSUM
accumulation across both experts in the down-projection, one row-unique
scatter. The shared-expert MLP is tensor-parallel over its intermediate dim
(256 of 2048 per core). Host does only the tiny routing metadata (softmax over
16 logits + top-3-of-8 group pick) and the final 8-way partial sum; all GEMMs,
gather/scatter and silu run on device in float32r (full-rate fp32 path).

Weights are re-laid-out on the host at sharding time so every contraction dim
lands on SBUF partitions (PE requires partition = contraction for both matmul
operands; fp32 cannot use DMA-transpose).
"""
import sys

sys.path.insert(0, "/opt/trn_rl_repo")

import numpy as np

import concourse.bass as bass
import concourse.mybir as mybir
import concourse.tile as tile
from concourse import bacc
from concourse.bass_utils import run_bass_kernel_spmd
from concourse.masks import make_identity

P = 128
M = 1024          # tokens
H = 1024          # hidden
NI = 1024         # moe_intermediate
E = 16
N_GROUP = 8
TOPK_GROUP = 3
I_SH = 2048       # shared-expert intermediate (n_shared * moe_intermediate)
ISH_C = I_SH // 8  # per-core shared slice = 256

F32 = mybir.dt.float32
F32R = mybir.dt.float32r
I32 = mybir.dt.int32
AF = mybir.ActivationFunctionType
MULT = mybir.AluOpType.mult

_PROGRAM_CACHE = {}


def _build_program(C):
    """SPMD program for one core; C = token capacity (multiple of 128, <=512)."""
    assert C % P == 0 and 0 < C <= 512
    NCH = C // P  # token chunks

    nc = bacc.Bacc("TRN2", target_bir_lowering=False, debug=False, num_devices=8)

    # --- per-core DRAM I/O ---
    w1s = nc.dram_tensor("w1s", [2, 16, P, 1024], F32R, kind="ExternalInput").ap()
    w2t = nc.dram_tensor("w2t", [2, 8, P, 1024], F32R, kind="ExternalInput").ap()
    ht = nc.dram_tensor("ht", [8, P, M], F32R, kind="ExternalInput").ap()
    hid = nc.dram_tensor("hid", [M, H], F32, kind="ExternalInput").ap()
    gus = nc.dram_tensor("gus", [8, P, 2 * ISH_C], F32R, kind="ExternalInput").ap()
    dst = nc.dram_tensor("dst", [2, P, H], F32R, kind="ExternalInput").ap()
    idxg = nc.dram_tensor("idxg", [P, NCH], I32, kind="ExternalInput").ap()
    idxs = nc.dram_tensor("idxs", [P, NCH], I32, kind="ExternalInput").ap()
    wab = nc.dram_tensor("wab", [2, P, C], F32, kind="ExternalInput").ap()
    routed = nc.dram_tensor("routed", [M + 1, H], F32, kind="ExternalOutput").ap()
    shared = nc.dram_tensor("shared", [M, H], F32, kind="ExternalOutput").ap()

    with tile.TileContext(nc) as tc:
        with (
            tc.tile_pool(name="const", bufs=1) as const,
            tc.tile_pool(name="persist", bufs=1) as persist,
            tc.tile_pool(name="stream", bufs=3) as stream,
            tc.tile_pool(name="work", bufs=2) as work,
            tc.tile_pool(name="psum", bufs=4, space="PSUM") as psum,
        ):
            ident = const.tile([P, P], F32, tag="ident")
            make_identity(nc, ident)

            t_idxg = const.tile([P, NCH], I32, tag="idxg")
            t_idxs = const.tile([P, NCH], I32, tag="idxs")
            nc.sync.dma_start(out=t_idxg[:], in_=idxg[:])
            nc.sync.dma_start(out=t_idxs[:], in_=idxs[:])
            t_wab = persist.tile([P, 2 * C], F32, tag="wab")
            for e in range(2):
                nc.sync.dma_start(out=t_wab[:, e * C:(e + 1) * C], in_=wab[e])

            # hidden^T resident: chunk k (h rows k*128..) at cols k*M
            t_ht = persist.tile([P, 8 * M], F32R, tag="ht")
            for k in range(8):
                nc.sync.dma_start(out=t_ht[:, k * M:(k + 1) * M], in_=ht[k])

            # --- gather routed tokens, transpose to XT ---
            t_xt = persist.tile([P, 8 * C], F32R, tag="xt")
            for j in range(NCH):
                xg = work.tile([P, H], F32, tag="xg")
                nc.gpsimd.indirect_dma_start(
                    out=xg[:],
                    out_offset=None,
                    in_=hid[:],
                    in_offset=bass.IndirectOffsetOnAxis(ap=t_idxg[:, j:j + 1], axis=0),
                )
                for k in range(8):
                    pt = psum.tile([P, 512], F32, space="PSUM", tag="mm")
                    nc.tensor.transpose(
                        out=pt[:, :P], in_=xg[:, k * P:(k + 1) * P], identity=ident[:]
                    )
                    nc.scalar.copy(
                        out=t_xt[:, k * C + j * P: k * C + (j + 1) * P], in_=pt[:, :P]
                    )

            # --- routed experts: GEMM1 + silu*up*weight -> gtw ---
            t_gtw = persist.tile([P, 2 * 8 * C], F32R, tag="gtw")  # [e][n-chunk][C]
            for e in range(2):
                for j in range(8):  # n-chunk (gate f-chunk j pairs with up f-chunk j+8)
                    w1g = stream.tile([P, 1024], F32R, tag="w1")
                    nc.sync.dma_start(out=w1g[:], in_=w1s[e, j])
                    w1u = stream.tile([P, 1024], F32R, tag="w1")
                    nc.sync.dma_start(out=w1u[:], in_=w1s[e, j + 8])
                    pg = psum.tile([P, 512], F32, space="PSUM", tag="mm")
                    pu = psum.tile([P, 512], F32, space="PSUM", tag="mm")
                    for k in range(8):
                        nc.tensor.matmul(
                            pg[:, :C],
                            w1g[:, k * P:(k + 1) * P],
                            t_xt[:, k * C:(k + 1) * C],
                            start=(k == 0),
                            stop=(k == 7),
                        )
                    for k in range(8):
                        nc.tensor.matmul(
                            pu[:, :C],
                            w1u[:, k * P:(k + 1) * P],
                            t_xt[:, k * C:(k + 1) * C],
                            start=(k == 0),
                            stop=(k == 7),
                        )
                    sg = work.tile([P, C], F32, tag="sg")
                    nc.scalar.activation(out=sg[:], in_=pg[:, :C], func=AF.Silu)
                    gt = work.tile([P, C], F32, tag="gt")
                    nc.vector.tensor_tensor(out=gt[:], in0=sg[:], in1=pu[:, :C], op=MULT)
                    nc.vector.tensor_tensor(
                        out=t_gtw[:, (e * 8 + j) * C:(e * 8 + j + 1) * C],
                        in0=gt[:],
                        in1=t_wab[:, e * C:(e + 1) * C],
                        op=MULT,
                    )

            # --- GEMM2: Y[c,o] = sum_e sum_n gtw_e[n,c] * w2t_e[n,o] ---
            # two o-half passes; 4 held PSUM accumulators (one per c-chunk)
            t_y = persist.tile([P, NCH * H], F32, tag="y")
            for oh in range(2):
                accs = [
                    psum.tile([P, 512], F32, space="PSUM", tag="acc", name=f"acc{oh}_{cc}")
                    for cc in range(NCH)
                ]
                for e in range(2):
                    for n in range(8):
                        w2 = stream.tile([P, 512], F32R, tag="w2")
                        nc.sync.dma_start(out=w2[:], in_=w2t[e, n, :, oh * 512:(oh + 1) * 512])
                        for cc in range(NCH):
                            nc.tensor.matmul(
                                accs[cc][:],
                                t_gtw[:, (e * 8 + n) * C + cc * P:(e * 8 + n) * C + (cc + 1) * P],
                                w2[:],
                                start=(e == 0 and n == 0),
                                stop=(e == 1 and n == 7),
                            )
                for cc in range(NCH):
                    nc.scalar.copy(
                        out=t_y[:, cc * H + oh * 512: cc * H + (oh + 1) * 512],
                        in_=accs[cc][:],
                    )

            # scatter Y rows to routed[token] (dummies go to trash row M)
            for cc in range(NCH):
                nc.gpsimd.indirect_dma_start(
                    out=routed[:],
                    out_offset=bass.IndirectOffsetOnAxis(ap=t_idxs[:, cc:cc + 1], axis=0),
                    in_=t_y[:, cc * H:(cc + 1) * H],
                    in_offset=None,
                )

            # --- shared expert (TP slice): GUT = [gate;up]^T @ ... ---
            t_gus = persist.tile([P, 8 * 2 * ISH_C], F32R, tag="gus")
            for k in range(8):
                nc.sync.dma_start(
                    out=t_gus[:, k * 2 * ISH_C:(k + 1) * 2 * ISH_C], in_=gus[k]
                )
            t_dst = persist.tile([P, 2 * H], F32R, tag="dst")
            for ip in range(2):
                nc.sync.dma_start(out=t_dst[:, ip * H:(ip + 1) * H], in_=dst[ip])

            t_gts = persist.tile([P, 2 * M], F32R, tag="gts")
            for ip in range(2):  # i-chunk pair (gate ip, up ip+2)
                for mh in range(2):  # m half
                    pg = psum.tile([P, 512], F32, space="PSUM", tag="mm")
                    pu = psum.tile([P, 512], F32, space="PSUM", tag="mm")
                    for k in range(8):
                        nc.tensor.matmul(
                            pg[:],
                            t_gus[:, k * 2 * ISH_C + ip * P: k * 2 * ISH_C + (ip + 1) * P],
                            t_ht[:, k * M + mh * 512: k * M + (mh + 1) * 512],
                            start=(k == 0),
                            stop=(k == 7),
                        )
                    for k in range(8):
                        nc.tensor.matmul(
                            pu[:],
                            t_gus[:, k * 2 * ISH_C + (2 + ip) * P: k * 2 * ISH_C + (3 + ip) * P],
                            t_ht[:, k * M + mh * 512: k * M + (mh + 1) * 512],
                            start=(k == 0),
                            stop=(k == 7),
                        )
                    ss = work.tile([P, 512], F32, tag="ss")
                    nc.scalar.activation(out=ss[:], in_=pg[:], func=AF.Silu)
                    nc.vector.tensor_tensor(
                        out=t_gts[:, ip * M + mh * 512: ip * M + (mh + 1) * 512],
                        in0=ss[:],
                        in1=pu[:],
                        op=MULT,
                    )

            # GEMM-s2: shared[m, o] = sum_i gts[i, m] * dst[i, o]
            for mc in range(8):
                os_t = work.tile([P, H], F32, tag="os")
                for oh in range(2):
                    ps = psum.tile([P, 512], F32, space="PSUM", tag="mm")
                    for ip in range(2):
                        nc.tensor.matmul(
                            ps[:],
                            t_gts[:, ip * M + mc * P: ip * M + (mc + 1) * P],
                            t_dst[:, ip * H + oh * 512: ip * H + (oh + 1) * 512],
                            start=(ip == 0),
                            stop=(ip == 1),
                        )
                    nc.scalar.copy(out=os_t[:, oh * 512:(oh + 1) * 512], in_=ps[:])
                nc.sync.dma_start(out=shared[mc * P:(mc + 1) * P, :], in_=os_t[:])

    nc.compile()
    return nc


def _get_program(C):
    if C not in _PROGRAM_CACHE:
        _PROGRAM_CACHE[C] = _build_program(C)
    return _PROGRAM_CACHE[C]


def _route(hidden_states, gate_w):
    """Numpy replica of grouped_topk: softmax -> per-group max -> top-3 groups.
    With E=16, n_group=8, topk_group=3, top_k=6, the top-6 experts are exactly
    all experts of the top-3 groups and keep their softmax scores."""
    lg = hidden_states @ gate_w.T
    lg = lg - lg.max(axis=1, keepdims=True)
    sc = np.exp(lg)
    sc /= sc.sum(axis=1, keepdims=True)
    gsc = sc.reshape(M, N_GROUP, E // N_GROUP).max(axis=2)
    top = np.argsort(-gsc, axis=1, kind="stable")[:, :TOPK_GROUP]
    gmask = np.zeros((M, N_GROUP), bool)
    np.put_along_axis(gmask, top, True, axis=1)
    return sc.astype(np.float32), gmask


def _prep_core(c, hidden, ht_l, w1, w2, sgu_t, sd_t, sc, gmask, C):
    NCH = C // P
    tok = np.nonzero(gmask[:, c])[0].astype(np.int32)
    n = len(tok)
    idxg = np.zeros(C, np.int32)
    idxg[:n] = tok
    idxs = np.full(C, M, np.int32)
    idxs[:n] = tok
    wa = np.zeros(C, np.float32)
    wb = np.zeros(C, np.float32)
    wa[:n] = sc[tok, 2 * c]
    wb[:n] = sc[tok, 2 * c + 1]

    w1s = np.empty((2, 16, P, 1024), np.float32)
    w2t = np.empty((2, 8, P, 1024), np.float32)
    for i, e in enumerate((2 * c, 2 * c + 1)):
        # block (f_chunk j, h_chunk k): [h_in (part), f_in] = w1[e][j*128+q, k*128+p]
        w1s[i] = (
            w1[e].reshape(16, P, 8, P).transpose(0, 3, 2, 1).reshape(16, P, 8 * P)
        )
        w2t[i] = np.ascontiguousarray(w2[e].T).reshape(8, P, 1024)

    gus = np.ascontiguousarray(
        np.concatenate(
            (
                sgu_t[:, c * ISH_C:(c + 1) * ISH_C],
                sgu_t[:, I_SH + c * ISH_C: I_SH + (c + 1) * ISH_C],
            ),
            axis=1,
        )
    ).reshape(8, P, 2 * ISH_C)
    dstc = np.ascontiguousarray(sd_t[c * ISH_C:(c + 1) * ISH_C, :]).reshape(2, P, H)

    return {
        "w1s": w1s,
        "w2t": w2t,
        "ht": ht_l,
        "hid": hidden,
        "gus": gus,
        "dst": dstc,
        "idxg": np.ascontiguousarray(idxg.reshape(NCH, P).T),
        "idxs": np.ascontiguousarray(idxs.reshape(NCH, P).T),
        "wab": np.ascontiguousarray(
            np.stack(
                (np.broadcast_to(wa, (P, C)), np.broadcast_to(wb, (P, C)))
            )
        ),
    }


def _run(inputs, trace=False):
    hidden = np.ascontiguousarray(np.asarray(inputs["hidden_states"], np.float32))
    gate_w = np.asarray(inputs["gate_w"], np.float32)
    w1 = np.asarray(inputs["w1"], np.float32)
    w2 = np.asarray(inputs["w2"], np.float32)
    sgu = np.asarray(inputs["shared_gate_up"], np.float32)
    sd = np.asarray(inputs["shared_down"], np.float32)

    sc, gmask = _route(hidden, gate_w)
    counts = gmask.sum(axis=0)
    C = int(min(512, max(P, -(-int(counts.max()) // P) * P)))
    assert counts.max() <= C, f"capacity overflow: {counts}"

    ht_l = np.ascontiguousarray(hidden.T).reshape(8, P, M)
    sgu_t = np.ascontiguousarray(sgu.T)  # [H, 2*I_SH]
    sd_t = np.ascontiguousarray(sd.T)    # [I_SH, H]

    nc = _get_program(C)
    in_maps = [
        _prep_core(c, hidden, ht_l, w1, w2, sgu_t, sd_t, sc, gmask, C)
        for c in range(8)
    ]
    res = run_bass_kernel_spmd(
        nc, in_maps, core_ids=list(range(8)), trace=trace
    )
    out = np.zeros((M, H), np.float32)
    for c in range(8):
        out += res.results[c]["routed"][:M]
        out += res.results[c]["shared"]
    return out, res


def kernel(**inputs):
    out, _ = _run(inputs, trace=False)
    return out
